# revision 10
# baseline (speedup 1.0000x reference)
"""Two-layer DGL-style GCN (norm='both') on 8 TRN2 NeuronCores.

v3 design (vs v2):
  * The z-exchange table is block-interleaved: local tiles are split into
    blocks (each block's rows, concatenated over cores, form one int16
    gather range <= 32768 rows).  Each block is exchanged with its own
    sub-AllGather as soon as layer 1 finishes those tiles, so layer-2
    gather descriptor generation (the Q7 bottleneck, ~8.6ns/edge) starts
    ~30us into the kernel instead of after the whole of layer 1.
  * Layer 2 runs range-major: for each range, all groups' slots are
    gathered and partially aggregated into an SBUF fp32 accumulator;
    the final range's partial is fused with projection/output.
  * Layer 1 consumes host-expanded edge rows + 0/1 masks (shared by both
    layers) in small 2-tile windows with PSUM accumulation.

kernel(**inputs) takes the full unsharded inputs and returns the full
output; all sharding happens inside.
"""

import math

import numpy as np

import concourse.bacc as bacc
import concourse.bass as bass
import concourse.bass_utils as bass_utils
import concourse.mybir as mybir
import concourse.tile as tile

P = 128
RR = 32768  # rows addressable by one int16-indexed gather range

N_NODES = 100000
N_EDGES = 1600000
C_IN = 128
C_HID = 128
C_OUT = 40
N_CORES = 8

F16 = mybir.dt.float16
F32 = mybir.dt.float32
I16 = mybir.dt.int16

TRACE = False
LAST_RESULTS = None


def _cdiv(a, b):
    return -(-a // b)


def _make_blocks(ntiles, maxblk):
    """Split tiles into blocks, each <= maxblk; small first block so the
    first sub-AllGather (and layer-2 descriptor gen) can start early."""
    if ntiles <= maxblk:
        return [ntiles]
    nfull = (ntiles - 1) // maxblk
    b0 = ntiles - nfull * maxblk
    return [b0] + [maxblk] * nfull


# ---------------------------------------------------------------- host prep


def prep_inputs(x, edge_index, W1, W2, ncores):
    n, cin = x.shape
    chid = W1.shape[1]
    cout = W2.shape[1]
    e = edge_index.shape[1]

    ntiles_pc = math.ceil(n / (ncores * P))
    nb = ntiles_pc * P
    npad = nb * ncores

    maxblk = RR // (P * ncores)
    assert maxblk >= 1
    if ntiles_pc == 98 and maxblk == 32:
        blocks = [8, 26, 32, 32]
    else:
        blocks = _make_blocks(ntiles_pc, maxblk)
    nr = len(blocks)
    blk_tile_base = np.concatenate([[0], np.cumsum(blocks)])  # [nr+1]

    gt = 1
    for cand in range(min(7, ntiles_pc), 0, -1):
        if ntiles_pc % cand == 0:
            gt = cand
            break
    ng = ntiles_pc // gt

    src = np.asarray(edge_index[0], dtype=np.int64)
    dst = np.asarray(edge_index[1], dtype=np.int64)

    deg_out = np.bincount(src, minlength=npad).astype(np.float32)
    deg_in = np.bincount(dst, minlength=npad).astype(np.float32)
    oi = 1.0 / np.sqrt(np.maximum(deg_out, 1.0))
    ii = 1.0 / np.sqrt(np.maximum(deg_in, 1.0))
    s1 = ii * oi
    s2 = ii

    x16 = np.zeros((npad, cin), dtype=np.float16)
    x16[:n] = np.asarray(x, dtype=np.float32) * oi[:n, None]

    # src's block (range) + position in the block-interleaved table
    s_owner = src // nb
    s_local = src % nb
    s_tile = s_local // P
    blk_of_tile = np.searchsorted(blk_tile_base[1:], np.arange(ntiles_pc), "right")
    s_blk = blk_of_tile[s_tile]
    blk_rows = np.asarray(blocks) * P
    idx_in_rng = s_owner * blk_rows[s_blk] + (s_local - blk_tile_base[s_blk] * P)
    assert idx_in_rng.max() < RR

    # bucket edges by (core, dst tile, src block); sort by src within bucket
    core = dst // nb
    tl = (dst % nb) // P
    key = (core * ntiles_pc + tl) * nr + s_blk
    order = np.lexsort((src, key))
    ks = key[order]
    idx_s = idx_in_rng[order]
    srcs = src[order]
    dsts = dst[order]
    dl = (dsts % P).astype(np.int64)

    nbuck = ncores * ntiles_pc * nr
    cnt = np.bincount(ks, minlength=nbuck).reshape(ncores, ntiles_pc, nr)
    S = _cdiv(cnt.max(axis=0), P)  # [ntiles_pc, nr]
    S = np.maximum(S, 1)
    NCOL = int(S.sum())
    NF = NCOL * P

    # stream order: for g: for r: for t in g
    BS = np.zeros((ntiles_pc, nr), dtype=np.int64)
    chunk_base = np.zeros((ng, nr), dtype=np.int64)
    chunk_n = np.zeros((ng, nr), dtype=np.int64)
    pos = 0
    for g in range(ng):
        for r in range(nr):
            chunk_base[g, r] = pos
            for t in range(g * gt, (g + 1) * gt):
                BS[t, r] = pos
                pos += int(S[t, r]) * P
            chunk_n[g, r] = pos - chunk_base[g, r]
    assert pos == NF

    starts = np.zeros(nbuck + 1, dtype=np.int64)
    starts[1:] = np.cumsum(cnt.reshape(-1))
    posin = np.arange(e, dtype=np.int64) - starts[ks]
    t_arr = (ks // nr) % ntiles_pc
    r_arr = ks % nr
    c_arr = ks // (ntiles_pc * nr)
    F = BS[t_arr, r_arr] + posin

    xg_l, idx_l, s1_l, s2_l = [], [], [], []
    W1_16 = np.asarray(W1, dtype=np.float16)
    W2p = np.zeros((chid, P), dtype=np.float16)
    W2p[:, :cout] = np.asarray(W2, dtype=np.float16)
    ident = np.eye(P, dtype=np.float16)

    for c in range(ncores):
        m = c_arr == c
        Fc = F[m]

        # fused per-column [edge-row | mask] stream
        xmf = np.zeros((NF, cin + P), dtype=np.float16)
        xmf[Fc, :cin] = x16[srcs[m]]
        xmf[Fc, cin + dl[m]] = 1.0
        xm = (
            xmf.reshape(NCOL, P, cin + P)
            .transpose(1, 0, 2)
            .reshape(P, NCOL * (cin + P))
            .copy()
        )
        del xmf

        idxv = np.zeros(NF, dtype=np.int16)
        idxv[Fc] = idx_s[m].astype(np.int16)
        idx16 = np.tile(idxv.reshape(NF // 16, 16).transpose(1, 0), (8, 1)).copy()

        nodes = c * nb + np.arange(nb)
        s1_l.append(s1[nodes].reshape(ntiles_pc, P).T.astype(np.float32).copy())
        s2_l.append(s2[nodes].reshape(ntiles_pc, P).T.astype(np.float32).copy())
        xg_l.append(xm)
        idx_l.append(idx16)

    in_maps = [
        {
            "xm": xg_l[c],
            "idx16": idx_l[c],
            "s1": s1_l[c],
            "s2": s2_l[c],
            "W1": W1_16,
            "W2": W2p,
            "ident": ident,
        }
        for c in range(ncores)
    ]

    meta = dict(
        n=n, cin=cin, chid=chid, cout=cout,
        ncores=ncores, ntiles_pc=ntiles_pc, nb=nb, npad=npad,
        nr=nr, gt=gt, ng=ng,
        blocks=tuple(blocks),
        S=tuple(tuple(int(v) for v in row) for row in S),
        NCOL=NCOL,
        chunk_base=tuple(tuple(int(v) for v in row) for row in chunk_base),
        chunk_n=tuple(tuple(int(v) for v in row) for row in chunk_n),
        BS=tuple(tuple(int(v) for v in row) for row in BS),
    )
    return in_maps, meta


# ---------------------------------------------------------------- device program


def build_nc(meta, debug=False, enable_asserts=False):
    cin = meta["cin"]
    chid = meta["chid"]
    ncores = meta["ncores"]
    ntiles_pc = meta["ntiles_pc"]
    nb = meta["nb"]
    npad = meta["npad"]
    nr = meta["nr"]
    gt = meta["gt"]
    ng = meta["ng"]
    blocks = meta["blocks"]
    S = meta["S"]
    NCOL = meta["NCOL"]
    chunk_base = meta["chunk_base"]
    chunk_n = meta["chunk_n"]
    BS = meta["BS"]
    NF = NCOL * P

    blk_tile_base = [0]
    for b in blocks:
        blk_tile_base.append(blk_tile_base[-1] + b)
    rng_row_base = [blk_tile_base[r] * P * ncores for r in range(nr)]
    rng_rows = [blocks[r] * P * ncores for r in range(nr)]

    # layer-1 windows: pairs of tiles, never spanning a group boundary
    w1 = 2 if gt % 2 == 0 or gt == 1 else 2
    windows = []
    for g in range(ng):
        t = g * gt
        while t < (g + 1) * gt:
            wt = min(3, (g + 1) * gt - t)
            windows.append((t, wt))
            t += wt

    def win_after(tile_idx):
        for i, (t0, wt) in enumerate(windows):
            if t0 + wt - 1 >= tile_idx:
                return i
        return len(windows) - 1

    ag_after = [win_after(blk_tile_base[r + 1] - 1) for r in range(nr)]
    # interleave range-0 layer-2 chunks among layer-1 windows
    l2r0_after = {}
    if nr > 1:
        for g in range(ng):
            w = min(ag_after[0] + 1 + 2 * g, len(windows) - 1)
            l2r0_after.setdefault(w, []).append(g)

    WCOL = max(
        sum(S[t0 + k][r] for k in range(wt) for r in range(nr)) for t0, wt in windows
    )
    CCOL = max(chunk_n[g][r] // P for g in range(ng) for r in range(nr))
    R0COL = max(chunk_n[g][0] // P for g in range(ng))

    nc = bacc.Bacc(
        "TRN2",
        target_bir_lowering=False,
        debug=debug,
        enable_asserts=enable_asserts,
        num_devices=ncores,
    )

    CW = cin + P  # fused column width (edge row | mask)
    xm_d = nc.dram_tensor("xm", [P, NCOL * CW], F16, kind="ExternalInput")
    idx_d = nc.dram_tensor("idx16", [P, NF // 16], I16, kind="ExternalInput")
    s1_d = nc.dram_tensor("s1", [P, ntiles_pc], F32, kind="ExternalInput")
    s2_d = nc.dram_tensor("s2", [P, ntiles_pc], F32, kind="ExternalInput")
    W1 = nc.dram_tensor("W1", [cin, chid], F16, kind="ExternalInput")
    W2 = nc.dram_tensor("W2", [chid, P], F16, kind="ExternalInput")
    ident_d = nc.dram_tensor("ident", [P, P], F16, kind="ExternalInput")

    out = nc.dram_tensor("out", [nb, P], F32, kind="ExternalOutput")

    H2b = nc.dram_tensor("H2b", [nb, chid], F16)
    H2f = nc.dram_tensor("H2f", [npad, chid], F16, addr_space="Shared")

    with tile.TileContext(nc) as tc:
        with (
            tc.tile_pool(name="const", bufs=1) as cpool,
            tc.tile_pool(name="w_rows", bufs=2) as wrpool,
            tc.tile_pool(name="g2buf", bufs=3) as g2pool,
            tc.tile_pool(name="g2r0", bufs=8) as g2r0pool,
            tc.tile_pool(name="c_mask", bufs=2) as cmpool,
            tc.tile_pool(name="c_idx", bufs=3) as cipool,
            tc.tile_pool(name="agg_ps", bufs=4, space="PSUM") as pspool,
            tc.tile_pool(name="aux_ps", bufs=2, space="PSUM") as xpspool,
            tc.tile_pool(name="flush", bufs=4) as flpool,
        ):
            w1_sb = cpool.tile([cin, chid], F16)
            nc.sync.dma_start(w1_sb[:], W1.ap())
            w2_sb = cpool.tile([chid, P], F16)
            nc.sync.dma_start(w2_sb[:], W2.ap())
            ident_f = cpool.tile([P, P], F16)
            nc.sync.dma_start(ident_f[:], ident_d.ap())
            s1_sb = cpool.tile([P, ntiles_pc], F32)
            nc.sync.dma_start(s1_sb[:], s1_d.ap())
            s2_sb = cpool.tile([P, ntiles_pc], F32)
            nc.sync.dma_start(s2_sb[:], s2_d.ap())
            acc = (
                cpool.tile([P, nb], F16, name="acc") if nr > 1 else None
            )
            idx0 = None
            idx0_off = {}
            if nr > 1:
                idx0 = cpool.tile([P, ng * R0COL * P // 16], I16, name="idx0")
                o = 0
                for g in range(ng):
                    n0 = chunk_n[g][0]
                    idx0_off[g] = o
                    nc.sync.dma_start(
                        idx0[:, o : o + n0 // 16],
                        idx_d.ap()[
                            :, chunk_base[g][0] // 16 : (chunk_base[g][0] + n0) // 16
                        ],
                    )
                    o += n0 // 16

            def emit_out_tile(t, a2s):
                o2 = xpspool.tile([P, P], F32, tag="proj")
                nc.tensor.matmul(
                    o2[:], lhsT=w2_sb[:], rhs=a2s[:], start=True, stop=True
                )
                o2s = flpool.tile([P, P], F16, tag="o2s")
                nc.vector.tensor_copy(o2s[:], o2[:])
                o2t = xpspool.tile([P, P], F16, tag="tr")
                nc.tensor.transpose(o2t[:], o2s[:], ident_f[:])
                os = flpool.tile([P, P], F32, tag="os")
                nc.vector.tensor_scalar(
                    out=os[:],
                    in0=o2t[:],
                    scalar1=s2_sb[:, t : t + 1],
                    scalar2=None,
                    op0=mybir.AluOpType.mult,
                )
                nc.sync.dma_start(out.ap()[t * P : (t + 1) * P, :], os[:])

            def emit_l1_window(t0, wt):
                rbase = []
                for r in range(nr):
                    c0 = BS[t0][r] // P
                    ncols = sum(S[t0 + k][r] for k in range(wt))
                    rbase.append((c0, ncols))
                XW = wrpool.tile([P, WCOL * CW], F16, tag="xw")
                off = 0
                woff = []
                for r, (c0, ncols) in enumerate(rbase):
                    woff.append(off)
                    nc.sync.dma_start(
                        XW[:, off * CW : (off + ncols) * CW],
                        xm_d.ap()[:, c0 * CW : (c0 + ncols) * CW],
                    )
                    off += ncols
                for k in range(wt):
                    t = t0 + k
                    ncols_t = sum(S[t][r] for r in range(nr))
                    at = pspool.tile([P, P], F32, tag="at")
                    j = 0
                    for r in range(nr):
                        base = woff[r] + sum(S[t0 + kk][r] for kk in range(k))
                        for cc in range(S[t][r]):
                            cl = base + cc
                            nc.tensor.matmul(
                                at[:],
                                lhsT=XW[:, cl * CW : cl * CW + cin],
                                rhs=XW[:, cl * CW + cin : (cl + 1) * CW],
                                start=(j == 0),
                                stop=(j == ncols_t - 1),
                            )
                            j += 1
                    ats = flpool.tile([P, P], F16, tag="ats")
                    nc.vector.tensor_copy(ats[:], at[:])
                    y1 = xpspool.tile([P, P], F32, tag="proj")
                    nc.tensor.matmul(
                        y1[:], lhsT=w1_sb[:], rhs=ats[:], start=True, stop=True
                    )
                    x2 = flpool.tile([P, P], F16, tag="x2")
                    nc.scalar.activation(
                        x2[:], y1[:], mybir.ActivationFunctionType.Relu
                    )
                    zt = xpspool.tile([P, P], F16, tag="tr")
                    nc.tensor.transpose(zt[:], x2[:], ident_f[:])
                    zs = flpool.tile([P, P], F16, tag="zs")
                    nc.vector.tensor_scalar(
                        out=zs[:],
                        in0=zt[:],
                        scalar1=s1_sb[:, t : t + 1],
                        scalar2=None,
                        op0=mybir.AluOpType.mult,
                    )
                    nc.sync.dma_start(H2b.ap()[t * P : (t + 1) * P, :], zs[:])

            def emit_ag(r):
                t0, t1 = blk_tile_base[r], blk_tile_base[r + 1]
                nc.gpsimd.collective_compute(
                    "AllGather",
                    mybir.AluOpType.bypass,
                    replica_groups=[list(range(ncores))],
                    ins=[H2b.ap()[t0 * P : t1 * P, :].opt()],
                    outs=[
                        H2f.ap()[
                            rng_row_base[r] : rng_row_base[r] + rng_rows[r], :
                        ].opt()
                    ],
                )

            def emit_l2_chunk(g, r):
                nidx = chunk_n[g][r]
                ccols = nidx // P
                c0 = chunk_base[g][r] // P
                if r == 0 and nr > 1:
                    G2 = g2r0pool.tile([P, R0COL * chid], F16, tag="g2r0")
                    idx_ap = idx0[:, idx0_off[g] : idx0_off[g] + nidx // 16]
                else:
                    G2 = g2pool.tile([P, CCOL * chid], F16, tag="g2")
                    IDX = cipool.tile([P, _cdiv(CCOL * P, 16)], I16, tag="cidx")
                    nc.scalar.dma_start(
                        IDX[:, : nidx // 16],
                        idx_d.ap()[
                            :,
                            chunk_base[g][r] // 16 : (chunk_base[g][r] + nidx) // 16,
                        ],
                    )
                    idx_ap = IDX[:, : nidx // 16]
                nc.gpsimd.dma_gather(
                    out_ap=G2[:, : ccols * chid].rearrange("p (c e) -> p c e", e=chid),
                    in_ap=H2f.ap()[rng_row_base[r] : rng_row_base[r] + rng_rows[r], :],
                    idxs_ap=idx_ap,
                    num_idxs=nidx,
                    num_idxs_reg=nidx,
                    elem_size=chid,
                    single_packet=False,
                )
                MC = cmpool.tile([P, CCOL * CW], F16, tag="cmk")
                nc.scalar.dma_start(
                    MC[:, : ccols * CW], xm_d.ap()[:, c0 * CW : (c0 + ccols) * CW]
                )
                off = 0
                for t in range(g * gt, (g + 1) * gt):
                    st = S[t][r]
                    at2 = pspool.tile([P, P], F32, tag="at")
                    for cc in range(st):
                        cl = off + cc
                        nc.tensor.matmul(
                            at2[:],
                            lhsT=G2[:, cl * chid : (cl + 1) * chid],
                            rhs=MC[:, cl * CW + cin : (cl + 1) * CW],
                            start=(cc == 0),
                            stop=(cc == st - 1),
                        )
                    off += st
                    sl = slice(t * P, (t + 1) * P)
                    if nr == 1:
                        a2s = flpool.tile([P, P], F16, tag="a2s")
                        nc.vector.tensor_copy(a2s[:], at2[:])
                        emit_out_tile(t, a2s)
                    elif r == 0:
                        nc.vector.tensor_copy(acc[:, sl], at2[:])
                    elif r < nr - 1:
                        nc.vector.tensor_tensor(
                            out=acc[:, sl],
                            in0=acc[:, sl],
                            in1=at2[:],
                            op=mybir.AluOpType.add,
                        )
                    else:
                        a2s = flpool.tile([P, P], F16, tag="a2s")
                        nc.vector.tensor_tensor(
                            out=a2s[:],
                            in0=acc[:, sl],
                            in1=at2[:],
                            op=mybir.AluOpType.add,
                        )
                        emit_out_tile(t, a2s)

            # ---- emission schedule
            for i, (t0, wt) in enumerate(windows):
                emit_l1_window(t0, wt)
                for r in range(nr):
                    if ag_after[r] == i:
                        emit_ag(r)
                for g in l2r0_after.get(i, []):
                    emit_l2_chunk(g, 0)
            if nr == 1:
                for g in range(ng):
                    emit_l2_chunk(g, 0)
            else:
                for r in range(1, nr):
                    for g in range(ng):
                        emit_l2_chunk(g, r)

    nc.compile()
    return nc


# ---------------------------------------------------------------- entry point

_CACHE = {}


def kernel(x, edge_index, W1, W2):
    global LAST_RESULTS
    x = np.asarray(x)
    edge_index = np.asarray(edge_index)
    W1 = np.asarray(W1)
    W2 = np.asarray(W2)

    in_maps, meta = prep_inputs(x, edge_index, W1, W2, N_CORES)

    key = (meta["npad"], meta["S"], meta["gt"])
    nc = _CACHE.get(key)
    if nc is None:
        nc = build_nc(meta, debug=False, enable_asserts=False)
        _CACHE[key] = nc

    res = bass_utils.run_bass_kernel_spmd(
        nc,
        in_maps,
        core_ids=list(range(meta["ncores"])),
        trace=TRACE,
    )
    LAST_RESULTS = res

    blocks_ = [res.results[c]["out"] for c in range(meta["ncores"])]
    full = np.concatenate(blocks_, axis=0)  # [npad, P]
    return np.ascontiguousarray(full[: meta["n"], : meta["cout"]]).astype(np.float32)


# revision 13
# speedup vs baseline: 1.2250x; 1.2250x over previous
"""Two-layer DGL-style GCN (norm='both') on 8 TRN2 NeuronCores.

v2 design (vs baseline):
  * Per-edge scales are folded into per-dst-node scales applied after
    aggregation, so the one-hot "routing" matrices are pure 0/1 and
    IDENTICAL for both layers.  They are built on the host and streamed
    from DRAM -- zero Vector/GpSimd per-column work on device.
  * Layer 1 edge rows are expanded on the host into slot order
    (halo-replication done during sharding), so layer 1 is a pure
    sequential stream + PE matmuls: no device gather.
  * Layer 2 gathers z = oi*relu(h1) rows per edge slot from an
    AllGather'ed table with gpsimd.dma_gather (the only Q7 work).
  * Aggregation stays one-hot matmul on PE with PSUM accumulation;
    projection (W1 / W2) after aggregation; per-dst scales applied with
    per-partition tensor_scalar after a PE transpose.

kernel(**inputs) takes the full unsharded inputs and returns the full
output; all sharding happens inside.
"""

import math

import numpy as np

import concourse.bacc as bacc
import concourse.bass as bass
import concourse.bass_utils as bass_utils
import concourse.mybir as mybir
import concourse.tile as tile

P = 128
RR = 32768  # rows addressable by one int16-indexed gather range

# Full-problem constants (the grading harness calls kernel() with these shapes)
N_NODES = 100000
N_EDGES = 1600000
C_IN = 128
C_HID = 128
C_OUT = 40
N_CORES = 8

F16 = mybir.dt.float16
F8 = mybir.dt.float8e4
F32 = mybir.dt.float32
I16 = mybir.dt.int16

# set by test.py to request a profiled run
TRACE = False
LAST_RESULTS = None


def _cdiv(a, b):
    return -(-a // b)


# ---------------------------------------------------------------- host prep


def prep_inputs(x, edge_index, W1, W2, ncores):
    """Shard the full inputs -> (in_maps, meta)."""
    n, cin = x.shape
    chid = W1.shape[1]
    cout = W2.shape[1]
    e = edge_index.shape[1]

    ntiles_pc = math.ceil(n / (ncores * P))  # dst tiles per core
    nb = ntiles_pc * P  # dst nodes per core
    npad = nb * ncores
    nr = _cdiv(npad, RR)  # int16 gather ranges

    # group of dst tiles processed per streamed chunk (must divide ntiles_pc)
    gt = 1
    for cand in range(min(7, ntiles_pc), 0, -1):
        if ntiles_pc % cand == 0:
            gt = cand
            break
    ng = ntiles_pc // gt

    src = np.asarray(edge_index[0], dtype=np.int64)
    dst = np.asarray(edge_index[1], dtype=np.int64)

    deg_out = np.bincount(src, minlength=npad).astype(np.float32)
    deg_in = np.bincount(dst, minlength=npad).astype(np.float32)
    oi = 1.0 / np.sqrt(np.maximum(deg_out, 1.0))
    ii = 1.0 / np.sqrt(np.maximum(deg_in, 1.0))
    s1 = ii * oi  # post-L1 per-dst scale (ii for conv1, oi pre-folded for L2)
    s2 = ii  # post-L2 per-dst scale

    # out-degree scale folded into the node feature table
    x16 = np.zeros((npad, cin), dtype=np.float16)
    x16[:n] = np.asarray(x, dtype=np.float32) * oi[:n, None]

    # bucket edges by (core, dst tile, src range); sort by src within bucket
    core = dst // nb
    tl = (dst % nb) // P
    rng = src // RR
    key = (core * ntiles_pc + tl) * nr + rng
    order = np.lexsort((src, key))
    ks = key[order]
    srcs = src[order]
    dsts = dst[order]
    dl = (dsts % P).astype(np.int64)  # dst lane within tile

    nbuck = ncores * ntiles_pc * nr
    cnt = np.bincount(ks, minlength=nbuck).reshape(ncores, ntiles_pc, nr)
    # per-(tile,range) column count: max over cores (SPMD uniform program)
    S = _cdiv(cnt.max(axis=0), P)  # [ntiles_pc, nr]
    S = np.maximum(S, 1)
    cols_t = S.sum(axis=1)  # [ntiles_pc]
    NCOL = int(cols_t.sum())
    NF = NCOL * P

    # slot stream order: for each group g: for r in ranges: for t in group:
    #   S[t,r]*P slots.  Compute base slot offset per (t, r).
    BS = np.zeros((ntiles_pc, nr), dtype=np.int64)
    pos = 0
    chunk_base = np.zeros((ng, nr), dtype=np.int64)  # slot base of (g, r)
    chunk_n = np.zeros((ng, nr), dtype=np.int64)  # slots in (g, r)
    for g in range(ng):
        for r in range(nr):
            chunk_base[g, r] = pos
            for t in range(g * gt, (g + 1) * gt):
                BS[t, r] = pos
                pos += int(S[t, r]) * P
            chunk_n[g, r] = pos - chunk_base[g, r]
    assert pos == NF

    # flat slot id per edge (within its core's stream)
    starts = np.zeros(nbuck + 1, dtype=np.int64)
    starts[1:] = np.cumsum(cnt.reshape(-1))
    posin = np.arange(e, dtype=np.int64) - starts[ks]
    t_arr = (ks // nr) % ntiles_pc
    r_arr = ks % nr
    c_arr = ks // (ntiles_pc * nr)
    F = BS[t_arr, r_arr] + posin

    # per-core structures
    xg_l, mk_l, idx_l, s1_l, s2_l = [], [], [], [], []
    W1_16 = np.asarray(W1, dtype=np.float16)
    W2p = np.zeros((chid, P), dtype=np.float16)
    W2p[:, :cout] = np.asarray(W2, dtype=np.float16)
    ident = np.eye(P, dtype=np.float16)

    for c in range(ncores):
        m = c_arr == c
        Fc = F[m]
        srcc = srcs[m]
        dlc = dl[m]

        # layer-1 expanded edge rows, slot-major: xg[p, col*P + f]
        xgf = np.zeros((NF, cin), dtype=np.float16)
        xgf[Fc] = x16[srcc]
        xg = (
            xgf.reshape(NCOL, P, cin).transpose(1, 0, 2).reshape(P, NCOL * cin).copy()
        )
        del xgf

        # 0/1 routing mask, slot-major: mk[p, col*P + j]
        import ml_dtypes
        mkf = np.zeros((NF, P), dtype=ml_dtypes.float8_e4m3fn)
        mkf[Fc, dlc] = 1.0
        mk = mkf.reshape(NCOL, P, P).transpose(1, 0, 2).reshape(P, NCOL * P).copy()
        del mkf

        # layer-2 gather indices (int16 within range), slot order
        idxv = np.zeros(NF, dtype=np.int16)
        idxv[Fc] = (srcc - r_arr[m] * RR).astype(np.int16)
        idx16 = np.tile(
            idxv.reshape(NF // 16, 16).transpose(1, 0), (8, 1)
        ).copy()  # [128, NF//16]

        # per-dst-node scales: [lane, tile]
        nodes = c * nb + np.arange(nb)
        s1_t = s1[nodes].reshape(ntiles_pc, P).T.astype(np.float32).copy()
        s2_t = s2[nodes].reshape(ntiles_pc, P).T.astype(np.float32).copy()

        xg_l.append(xg)
        mk_l.append(mk)
        idx_l.append(idx16)
        s1_l.append(s1_t)
        s2_l.append(s2_t)

    in_maps = [
        {
            "xg": xg_l[c],
            "mk": mk_l[c],
            "idx16": idx_l[c],
            "s1": s1_l[c],
            "s2": s2_l[c],
            "W1": W1_16,
            "W2": W2p,
            "ident": ident,
        }
        for c in range(ncores)
    ]

    meta = dict(
        n=n, cin=cin, chid=chid, cout=cout,
        ncores=ncores, ntiles_pc=ntiles_pc, nb=nb, npad=npad,
        nr=nr, gt=gt, ng=ng,
        S=tuple(tuple(int(v) for v in row) for row in S),
        NCOL=NCOL,
        chunk_base=tuple(tuple(int(v) for v in row) for row in chunk_base),
        chunk_n=tuple(tuple(int(v) for v in row) for row in chunk_n),
        BS=tuple(tuple(int(v) for v in row) for row in BS),
    )
    return in_maps, meta


# ---------------------------------------------------------------- device program


def build_nc(meta, debug=False, enable_asserts=False):
    cin = meta["cin"]
    chid = meta["chid"]
    ncores = meta["ncores"]
    ntiles_pc = meta["ntiles_pc"]
    nb = meta["nb"]
    npad = meta["npad"]
    nr = meta["nr"]
    gt = meta["gt"]
    ng = meta["ng"]
    S = meta["S"]
    NCOL = meta["NCOL"]
    chunk_base = meta["chunk_base"]
    chunk_n = meta["chunk_n"]
    BS = meta["BS"]
    NF = NCOL * P

    # per-group free-dim geometry (in columns)
    gcol0 = [chunk_base[g][0] // P for g in range(ng)]  # first col of group
    gncol = [
        (chunk_base[g + 1][0] // P if g + 1 < ng else NCOL) - gcol0[g]
        for g in range(ng)
    ]
    GW = max(gncol)  # columns per group buffer

    nc = bacc.Bacc(
        "TRN2",
        target_bir_lowering=False,
        debug=debug,
        enable_asserts=enable_asserts,
        num_devices=ncores,
    )

    xg_d = nc.dram_tensor("xg", [P, NCOL * cin], F16, kind="ExternalInput")
    mk_d = nc.dram_tensor("mk", [P, NCOL * P], F8, kind="ExternalInput")
    idx_d = nc.dram_tensor("idx16", [P, NF // 16], I16, kind="ExternalInput")
    s1_d = nc.dram_tensor("s1", [P, ntiles_pc], F32, kind="ExternalInput")
    s2_d = nc.dram_tensor("s2", [P, ntiles_pc], F32, kind="ExternalInput")
    W1 = nc.dram_tensor("W1", [cin, chid], F16, kind="ExternalInput")
    W2 = nc.dram_tensor("W2", [chid, P], F16, kind="ExternalInput")
    ident_d = nc.dram_tensor("ident", [P, P], F16, kind="ExternalInput")

    out = nc.dram_tensor("out", [nb, P], F32, kind="ExternalOutput")

    H2b = nc.dram_tensor("H2b", [nb, chid], F16)
    H2f = nc.dram_tensor("H2f", [npad, chid], F16, addr_space="Shared")

    with tile.TileContext(nc) as tc:
        with (
            tc.tile_pool(name="const", bufs=1) as cpool,
            tc.tile_pool(name="rows", bufs=2) as rowpool,
            tc.tile_pool(name="mkbuf", bufs=2) as mkpool,
            tc.tile_pool(name="agg_ps", bufs=2, space="PSUM") as pspool,
            tc.tile_pool(name="aux_ps", bufs=2, space="PSUM") as xpspool,
            tc.tile_pool(name="flush", bufs=4) as flpool,
        ):
            w1_sb = cpool.tile([cin, chid], F16)
            nc.sync.dma_start(w1_sb[:], W1.ap())
            w2_sb = cpool.tile([chid, P], F16)
            nc.sync.dma_start(w2_sb[:], W2.ap())
            ident_f = cpool.tile([P, P], F16)
            nc.sync.dma_start(ident_f[:], ident_d.ap())
            s1_sb = cpool.tile([P, ntiles_pc], F32)
            nc.sync.dma_start(s1_sb[:], s1_d.ap())
            s2_sb = cpool.tile([P, ntiles_pc], F32)
            nc.sync.dma_start(s2_sb[:], s2_d.ap())
            idx_all = cpool.tile([P, NF // 16], I16)
            nc.sync.dma_start(idx_all[:], idx_d.ap())

            def tile_cols(t):
                """column ids (global) for dst tile t, in stream order."""
                out_ = []
                for r in range(nr):
                    b = BS[t][r] // P
                    out_.extend(range(b, b + S[t][r]))
                return out_

            # ---- Layer 1: stream expanded rows + masks, aggregate, project
            for g in range(ng):
                c0, ncols = gcol0[g], gncol[g]
                XG = rowpool.tile([P, GW * cin], F16, tag="rows")
                nc.sync.dma_start(
                    XG[:, : ncols * cin], xg_d.ap()[:, c0 * cin : (c0 + ncols) * cin]
                )
                MK = mkpool.tile([P, GW * P], F8, tag="mk")
                nc.sync.dma_start(
                    MK[:, : ncols * P], mk_d.ap()[:, c0 * P : (c0 + ncols) * P]
                )
                for t in range(g * gt, (g + 1) * gt):
                    cols = tile_cols(t)
                    at = pspool.tile([P, P], F32, tag="at")  # [feat, dst]
                    for j, col in enumerate(cols):
                        cl = col - c0
                        nc.tensor.matmul(
                            at[:],
                            lhsT=XG[:, cl * cin : (cl + 1) * cin],
                            rhs=MK[:, cl * P : (cl + 1) * P],
                            start=(j == 0),
                            stop=(j == len(cols) - 1),
                        )
                    ats = flpool.tile([P, P], F16, tag="ats")
                    nc.vector.tensor_copy(ats[:], at[:])
                    y1 = xpspool.tile([P, P], F32, tag="proj")  # [hid, dst]
                    nc.tensor.matmul(
                        y1[:], lhsT=w1_sb[:], rhs=ats[:], start=True, stop=True
                    )
                    x2 = flpool.tile([P, P], F16, tag="x2")
                    nc.scalar.activation(
                        x2[:], y1[:], mybir.ActivationFunctionType.Relu
                    )
                    zt = xpspool.tile([P, P], F16, tag="tr")  # [dst, hid]
                    nc.tensor.transpose(zt[:], x2[:], ident_f[:])
                    zs = flpool.tile([P, P], F16, tag="zs")
                    nc.vector.tensor_scalar(
                        out=zs[:],
                        in0=zt[:],
                        scalar1=s1_sb[:, t : t + 1],
                        scalar2=None,
                        op0=mybir.AluOpType.mult,
                    )
                    nc.sync.dma_start(H2b.ap()[t * P : (t + 1) * P, :], zs[:])

            # ---- exchange z rows
            nc.gpsimd.collective_compute(
                "AllGather",
                mybir.AluOpType.bypass,
                replica_groups=[list(range(ncores))],
                ins=[H2b.ap().opt()],
                outs=[H2f.ap().opt()],
            )

            # ---- Layer 2: gather z rows per slot, aggregate, project
            for g in range(ng):
                c0, ncols = gcol0[g], gncol[g]
                G2 = rowpool.tile([P, GW * chid], F16, tag="rows")
                for r in range(nr):
                    nidx = chunk_n[g][r]
                    if nidx == 0:
                        continue
                    lo = r * RR
                    hi = min(npad, lo + RR)
                    cb = (chunk_base[g][r] - chunk_base[g][0]) // P  # local col
                    ib = chunk_base[g][r] // 16
                    nc.gpsimd.dma_gather(
                        out_ap=G2[:, cb * chid : (cb + nidx // P) * chid].rearrange(
                            "p (c e) -> p c e", e=chid
                        ),
                        in_ap=H2f.ap()[lo:hi, :],
                        idxs_ap=idx_all[:, ib : ib + nidx // 16],
                        num_idxs=nidx,
                        num_idxs_reg=nidx,
                        elem_size=chid,
                        single_packet=False,
                    )
                MK = mkpool.tile([P, GW * P], F8, tag="mk")
                nc.sync.dma_start(
                    MK[:, : ncols * P], mk_d.ap()[:, c0 * P : (c0 + ncols) * P]
                )
                for t in range(g * gt, (g + 1) * gt):
                    cols = tile_cols(t)
                    at2 = pspool.tile([P, P], F32, tag="at")  # [hid, dst]
                    for j, col in enumerate(cols):
                        cl = col - c0
                        nc.tensor.matmul(
                            at2[:],
                            lhsT=G2[:, cl * chid : (cl + 1) * chid],
                            rhs=MK[:, cl * P : (cl + 1) * P],
                            start=(j == 0),
                            stop=(j == len(cols) - 1),
                        )
                    a2s = flpool.tile([P, P], F16, tag="a2s")
                    nc.vector.tensor_copy(a2s[:], at2[:])
                    o2 = xpspool.tile([P, P], F32, tag="proj")  # [out, dst]
                    nc.tensor.matmul(
                        o2[:], lhsT=w2_sb[:], rhs=a2s[:], start=True, stop=True
                    )
                    o2s = flpool.tile([P, P], F16, tag="o2s")
                    nc.vector.tensor_copy(o2s[:], o2[:])
                    o2t = xpspool.tile([P, P], F16, tag="tr")  # [dst, out]
                    nc.tensor.transpose(o2t[:], o2s[:], ident_f[:])
                    os = flpool.tile([P, P], F32, tag="os")
                    nc.vector.tensor_scalar(
                        out=os[:],
                        in0=o2t[:],
                        scalar1=s2_sb[:, t : t + 1],
                        scalar2=None,
                        op0=mybir.AluOpType.mult,
                    )
                    nc.sync.dma_start(out.ap()[t * P : (t + 1) * P, :], os[:])

    nc.compile()
    return nc


# ---------------------------------------------------------------- entry point

_CACHE = {}


def kernel(x, edge_index, W1, W2):
    global LAST_RESULTS
    x = np.asarray(x)
    edge_index = np.asarray(edge_index)
    W1 = np.asarray(W1)
    W2 = np.asarray(W2)

    in_maps, meta = prep_inputs(x, edge_index, W1, W2, N_CORES)

    key = (meta["npad"], meta["S"], meta["gt"])
    nc = _CACHE.get(key)
    if nc is None:
        nc = build_nc(meta, debug=False, enable_asserts=False)
        _CACHE[key] = nc

    res = bass_utils.run_bass_kernel_spmd(
        nc,
        in_maps,
        core_ids=list(range(meta["ncores"])),
        trace=TRACE,
    )
    LAST_RESULTS = res

    # per-core out is [nb, P]; stitch and slice
    blocks = [res.results[c]["out"] for c in range(meta["ncores"])]
    full = np.concatenate(blocks, axis=0)  # [npad, P]
    return np.ascontiguousarray(full[: meta["n"], : meta["cout"]]).astype(np.float32)
